# revision 29
# baseline (speedup 1.0000x reference)
"""Trainium2 Bass kernel for DiagonalGMMPosterior (vq_codebook).

Reference computation (per batch b, descriptor n, cluster k):
    dist[k,n]  = sum_d (x[d,n] - mu_n[k,d])^2 * exp(-log_sigma[k,d])
    logits     = -dist + log_alpha[k] - 0.5 * sum_d log_sigma[k,d]
    out[k,n]   = softmax_k(logits)

Device strategy (8 NeuronCores, data-parallel over the batch axis):
  * Host folds all (K,D) parameter math into two GEMM weight matrices and a
    per-cluster constant, then CENTERS them across K.  Softmax is invariant
    to per-n shifts, so subtracting the K-mean of the logits (a rank-1
    update folded into the weights on host) bounds the logits to ~+-16 and
    removes the need for a per-n max reduction entirely.
  * x is pre-cast to fp16 on host (|x| ~ 5, rel err ~5e-4): halves the HBM
    read traffic (the memory roofline) and runs the PE at 1 cycle/row with
    cheap 2-byte weight loads (the baseline's f32r matmuls spent ~2x their
    stream time on 4-byte weight loads).
  * PACKED layout: K=64 only fills half the partitions, so each 512-column
    PSUM bank holds TWO consecutive 512-column blocks of n, one at
    partitions 0:64 (PE tile_position (0,0)) and one at 64:128 (tile
    position (0,64)).  Everything downstream of the distance GEMM (exp,
    denominator matmul, reciprocal, normalize-multiply, store) then runs at
    full 128-partition width, i.e. half the engine columns.
  * The host also folds a uniform -5 shift into the per-cluster constant
    (softmax-invariant) so exp() outputs fit fp16: the denominator matmul
    then runs at the PE's fast 16-bit rate and the normalize multiply
    reads a 2-byte operand.
  * Per core, per 2048-column super-tile of x (squares split between
    ScalarE and GpSimd so the DVE keeps capacity for the normalize):
       xsq  = x*x                              (ScalarE 1232 cols +
                                                GpSimd 816 cols, fp16)
       pd   = w1^T @ xsq + w2^T @ x            (TensorE, 8 packed matmuls
                                                into 2 PSUM banks)
       e    = exp(pd + cc - 5)                 (ScalarE, one [128,1024] op,
                                                fp16 out)
       pb   = ones_blockdiag^T @ e             (TensorE: per-block K-sum,
                                                broadcast to 64 partitions)
       r    = 1/pb                             (DVE fast approx reciprocal;
                                                sum >= 64*e^-5 > 0 always
                                                since logits are K-centered)
       out  = e * r -> fp16                    (DVE)
    x loads are [128, 4096] (two supers) to halve Sync-sequencer dispatch
    cost; stores write both packed partition halves of two supers with two
    DMAs via a rearranged DRAM access pattern.  Emission uses explicit
    keys: loads/squares keep a ~2-super lead over the GEMM but the
    schedule starts dense so the first exp is not gated on megabytes of
    DMA.
"""

import numpy as np

import concourse.bacc as bacc
import concourse.bass as bass
import concourse.tile as tile
from concourse import mybir
from concourse.bass_utils import run_bass_kernel_spmd

B, D, N, K = 16, 128, 16384, 64
NCORES = 8
BPC = B // NCORES  # batches per core
NT = 512           # one packed block (half a PSUM bank partition-wise)
SUP = 4 * NT       # compute super-tile: 2048 columns -> 2 packed PSUM banks
LSUP = 2 * SUP     # IO super-tile: one x load / one output store per 4096
SQ_ACT = 1232      # per-super square columns on ScalarE (rest on GpSimd)

F32 = mybir.dt.float32
F32R = mybir.dt.float32r
F16 = mybir.dt.float16
BF16 = mybir.dt.bfloat16

_CACHE = {}


def _build_nc():
    # Bacc (not raw Bass): its compile() pass legalizes Tile's multi-wait
    # instructions down to the 1-wait-per-instruction hardware limit.
    nc = bacc.Bacc("TRN2", target_bir_lowering=False, debug=False)
    x_in = nc.declare_dram_parameter("x", [BPC, D, N], F16, isOutput=False)
    w1_in = nc.declare_dram_parameter("w1", [D, K], F16, isOutput=False)
    w2_in = nc.declare_dram_parameter("w2", [D, K], F16, isOutput=False)
    cc_in = nc.declare_dram_parameter("cc", [2 * K, 1], F32, isOutput=False)
    ones_in = nc.declare_dram_parameter("ones_bd", [2 * K, 2 * K], F16, isOutput=False)
    # fp16 output halves the store traffic at ~5e-4 rounding (posteriors
    # live in [0,1]); the host widens back to fp32
    out_ext = nc.declare_dram_parameter("out", [BPC, K, N], F16, isOutput=True)

    with tile.TileContext(nc) as tc:
        with (
            tc.tile_pool(name="consts", bufs=1) as consts,
            tc.tile_pool(name="xp", bufs=5) as xp,
            tc.tile_pool(name="ep", bufs=8) as ep,
            tc.tile_pool(name="op", bufs=6) as op,
            tc.tile_pool(name="rp", bufs=8) as rp,
            tc.tile_pool(name="pd", bufs=2, space="PSUM") as pdp,
            tc.tile_pool(name="pb", bufs=2, space="PSUM") as pbp,
        ):
            # const tiles allocated up front; their DMAs are emitted from
            # the schedule AFTER the first two x loads so the input chain
            # (load -> square -> dist) starts ~4us earlier
            w1_sb = consts.tile([D, K], F16)
            w2_sb = consts.tile([D, K], F16)
            cc_sb = consts.tile([2 * K, 1], F32)
            ones_bd = consts.tile([2 * K, 2 * K], F16)

            def s_consts(i):
                nc.sync.dma_start(out=w1_sb, in_=w1_in[:, :])
                nc.sync.dma_start(out=w2_sb, in_=w2_in[:, :])
                nc.sync.dma_start(out=cc_sb, in_=cc_in[:, :])
                nc.sync.dma_start(out=ones_bd, in_=ones_in[:, :])

            def s_warm(i):
                # ~1.4us of short throwaway matmuls (w1 as both weight and
                # moving operand: w1 is the FIRST const to land, ~1.5us
                # before ones_bd) while the first x load is in flight:
                # ramps the PE p-state and retires right as the first xsq
                # becomes ready, so the first dist batch starts warm
                # without being blocked.
                wt = pdp.tile([2 * K, 2 * NT], F32, tag="pd")
                for _ in range(12):
                    nc.tensor.matmul(
                        wt[0:K, 0:K], w1_sb[:, :], w1_sb[:, :],
                        start=True, stop=True,
                    )

            n_sup = N // SUP  # 8 per batch row
            pairs = [(b, p) for b in range(BPC) for p in range(n_sup)]
            NP = len(pairs)
            st = [dict() for _ in range(NP)]

            # software-pipelined emission: each engine's in-order stream
            # interleaves stages of consecutive pairs so no stage
            # head-of-line-blocks the next pair's earlier stage
            def s0_load(i):
                # one [128, 4096] load feeds two compute super-tiles (fewer
                # 565 ns dispatches on the Sync sequencer); the first two
                # supers use small [128, 2048] loads so the pipeline's first
                # exp is not gated on megabytes of DMA
                b, p = pairs[i]
                n0 = p * SUP
                if i < 2:
                    xt = xp.tile([D, SUP], F16, tag="xt0")
                    nc.sync.dma_start(out=xt, in_=x_in[b, :, n0 : n0 + SUP])
                    st[i]["xt"] = xt
                    st[i]["xt_off"] = (xt, 0)
                elif i % 2 == 0:
                    xt = xp.tile([D, LSUP], F16, tag="xt")
                    # two half-tile DMAs: the first super's square becomes
                    # ready ~1.5us earlier and the per-queue burst halves
                    nc.sync.dma_start(out=xt[:, 0:SUP], in_=x_in[b, :, n0 : n0 + SUP])
                    nc.sync.dma_start(out=xt[:, SUP:LSUP], in_=x_in[b, :, n0 + SUP : n0 + LSUP])
                    st[i]["xt"] = xt[:, 0:SUP]
                    st[i]["xt_off"] = (xt, 0)
                else:
                    xt = st[i - 1]["xt_off"][0]
                    st[i]["xt"] = xt[:, SUP:LSUP]
                    st[i]["xt_off"] = (xt, SUP)

            def s1_square(i):
                # per-super square split between ScalarE (SQ_ACT cols) and
                # GpSimd (rest); the DVE carries reciprocal+normalize and
                # cannot also absorb the square
                xt, off = st[i]["xt_off"]
                if i < 2:
                    xsq = xp.tile([D, SUP], F16, tag="xsq0")
                    st[i]["xsq"] = xsq
                elif i % 2 == 0:
                    xsq = xp.tile([D, LSUP], F16, tag="xsq")
                    st[i]["xsq"] = xsq[:, 0:SUP]
                    st[i]["xsq_f"] = xsq
                else:
                    xsq = st[i - 1]["xsq_f"]
                    st[i]["xsq"] = xsq[:, SUP:LSUP]
                nc.scalar.activation(
                    out=xsq[:, off : off + SQ_ACT],
                    in_=xt[:, off : off + SQ_ACT],
                    func=mybir.ActivationFunctionType.Square,
                )
                nc.gpsimd.tensor_mul(
                    xsq[:, off + SQ_ACT : off + SUP],
                    xt[:, off + SQ_ACT : off + SUP],
                    xt[:, off + SQ_ACT : off + SUP],
                )

            def s2_dist(i):
                # pair-batched packed dist GEMM: one w1 residency covers 8
                # matmuls (both supers of the load pair), then one w2
                # residency - fewer stationary-weight switches and longer
                # uninterrupted PE runs (keeps the PE p-state high)
                if i % 2:
                    return
                pds = []
                for j in (i, i + 1):
                    pd_t = pdp.tile([2 * K, 2 * NT], F32, tag="pd")
                    st[j]["pd"] = pd_t
                    pds.append(j)
                if i == 0:
                    # first pair runs super-major so pd(0) completes without
                    # waiting on super 1's square: faster first exp
                    whj = [
                        (w, h, j)
                        for j in pds
                        for w in (w1_sb, w2_sb)
                        for h in range(2)
                    ]
                else:
                    whj = [
                        (w, h, j)
                        for w in (w1_sb, w2_sb)
                        for h in range(2)
                        for j in pds
                    ]
                # h (the PE tile position / PSUM partition half) varies
                # OUTSIDE g and j: consecutive matmuls keep the same
                # stationary weight AND position, so the PE skips the
                # ~100-177ns weight reload between them
                for w_sb, h, j in whj:
                    start = w_sb is w1_sb
                    src_t = st[j]["xsq"] if start else st[j]["xt"]
                    pd_t = st[j]["pd"]
                    for g in range(2):
                        c0 = g * 2 * NT + h * NT
                        nc.tensor.matmul(
                            pd_t[h * K : (h + 1) * K, g * NT : (g + 1) * NT],
                            w_sb[:, :],
                            src_t[:, c0 : c0 + NT],
                            start=start,
                            stop=not start,
                        )

            def s3_exp(i):
                pd_t = st[i].pop("pd")
                et = ep.tile([2 * K, 2 * NT], F16, tag="et")
                # one 1024-col op across both PSUM banks; bias is the packed
                # per-cluster constant (duplicated across the two halves)
                nc.scalar.activation(
                    out=et, in_=pd_t,
                    func=mybir.ActivationFunctionType.Exp,
                    bias=cc_sb, scale=1.0,
                )
                st[i]["et"] = et
                st[i].pop("xt")
                st[i].pop("xsq")

            def s4_den(i):
                et = st[i]["et"]
                # denominator: block-diagonal ones weight sums each packed
                # 64-partition block AND broadcasts the sum back to all 64
                # of its partitions in a single pass
                pb_t = pbp.tile([2 * K, 2 * NT], F32, tag="pb")
                for g in range(2):
                    sl = slice(g * NT, (g + 1) * NT)
                    nc.tensor.matmul(
                        pb_t[:, sl], ones_bd[:, :], et[:, sl],
                        start=True, stop=True,
                    )
                st[i]["pb"] = pb_t

            def s5_recip(i):
                pb_t = st[i].pop("pb")
                r_all = rp.tile([2 * K, 2 * NT], F32, tag="r")
                # ~18-bit-accurate custom-DVE reciprocal; the sum is always
                # >= 64*e^{-16} (K-centered logits), so the undefined edge
                # cases (0/denorm/inf) cannot occur
                nc.vector.reciprocal_approx_fast(out=r_all, in_=pb_t)
                st[i]["r"] = r_all

            def s6_mult(i):
                et, r_all = st[i].pop("et"), st[i].pop("r")
                if i % 2:
                    ot = st[i - 1]["ot_full"]
                    ov = ot[:, 2 * NT : 4 * NT]
                else:
                    ot = op.tile([2 * K, 4 * NT], F16, tag="ot")
                    st[i]["ot_full"] = ot
                    ov = ot[:, 0 : 2 * NT]
                nc.vector.tensor_mul(ov, et, r_all)

            def s7_store(i):
                # store once per two compute super-tiles; issued from the
                # GpSimd sequencer (cheap SWDGE dispatch) to keep the Sync
                # sequencer free for the x loads
                if i % 2 == 0:
                    return
                b, p = pairs[i]
                n0 = (p - 1) * SUP
                ot = st[i - 1].pop("ot_full")
                # DRAM view [h][k][q][c] <-> packed SBUF [h*64+k, q*512+c],
                # n = n0 + q*1024 + h*512 + c
                d4 = out_ext[b, :, n0 : n0 + LSUP].rearrange(
                    "k (q h c) -> h k q c", q=4, h=2
                )
                nc.sync.dma_start(out=d4[0], in_=ot[0:K, :])
                nc.sync.dma_start(out=d4[1], in_=ot[K : 2 * K, :])

            stages = [
                s0_load, s1_square, s2_dist, s3_exp,
                s4_den, s5_recip, s6_mult, s7_store,
            ]
            # explicit emission keys: loads/squares hold a steady ~2-super
            # lead over the GEMM (keeps every FIFO fed) but the schedule
            # starts dense, so the first exp is behind only two small
            # squares/loads instead of five 1 MB ones
            LEAD = [-4.15, -3.85, 0.0, 0.2, 0.3, 0.4, 0.5, 0.6]
            sched = []
            for i in range(NP):
                for k in range(len(stages)):
                    sched.append((i + LEAD[k], k, i))
            stages.append(s_consts)
            sched.append((-3.0, len(stages) - 1, 0))
            stages.append(s_warm)
            sched.append((-2.9, len(stages) - 1, 0))
            for _, k, i in sorted(sched):
                stages[k](i)
    nc.compile()
    return nc


def _host_params(mu, log_sigma, log_alpha):
    mu64 = mu.astype(np.float64)
    mu_n = mu64 / np.maximum(
        np.linalg.norm(mu64, axis=1, keepdims=True), 1e-12
    )
    sinv = np.exp(-log_sigma.astype(np.float64))  # (K, D)
    a1 = -sinv                                    # coeff of x^2 in logits
    a2 = 2.0 * mu_n * sinv                        # coeff of x
    c = (
        -np.sum(mu_n * mu_n * sinv, axis=1)
        + log_alpha.astype(np.float64)
        - 0.5 * np.sum(log_sigma.astype(np.float64), axis=1)
    )
    # center across K: softmax is invariant to per-n shifts, and this keeps
    # the on-device logits within exp()'s comfortable range (~+-16)
    a1c = a1 - a1.mean(axis=0, keepdims=True)
    a2c = a2 - a2.mean(axis=0, keepdims=True)
    ccv = c - c.mean() - 5.0
    w1 = np.ascontiguousarray(a1c.T, dtype=np.float16)  # (D, K)
    w2 = np.ascontiguousarray(a2c.T, dtype=np.float16)  # (D, K)
    cc = np.tile(ccv.astype(np.float32).reshape(K, 1), (2, 1))  # (128, 1)
    return w1, w2, cc


def _in_maps(x, mu, log_sigma, log_alpha):
    x = np.asarray(x).astype(np.float16)
    w1, w2, cc = _host_params(
        np.asarray(mu), np.asarray(log_sigma), np.asarray(log_alpha)
    )
    ones_bd = np.kron(np.eye(2), np.ones((K, K))).astype(np.float16)
    return [
        {
            "x": np.ascontiguousarray(x[i * BPC : (i + 1) * BPC]),
            "w1": w1,
            "w2": w2,
            "cc": cc,
            "ones_bd": ones_bd,
        }
        for i in range(NCORES)
    ]


def kernel(x, mu, log_sigma, log_alpha):
    if "nc" not in _CACHE:
        _CACHE["nc"] = _build_nc()
    nc = _CACHE["nc"]
    in_maps = _in_maps(x, mu, log_sigma, log_alpha)
    res = run_bass_kernel_spmd(nc, in_maps, list(range(NCORES))).results
    out = np.concatenate(
        [np.asarray(res[i]["out"]) for i in range(NCORES)], axis=0
    )
    return out.astype(np.float32)


# revision 31
# speedup vs baseline: 1.0093x; 1.0093x over previous
"""Trainium2 Bass kernel for DiagonalGMMPosterior (vq_codebook).

Reference computation (per batch b, descriptor n, cluster k):
    dist[k,n]  = sum_d (x[d,n] - mu_n[k,d])^2 * exp(-log_sigma[k,d])
    logits     = -dist + log_alpha[k] - 0.5 * sum_d log_sigma[k,d]
    out[k,n]   = softmax_k(logits)

Device strategy (8 NeuronCores, data-parallel over the batch axis):
  * Host folds all (K,D) parameter math into two GEMM weight matrices and a
    per-cluster constant, then CENTERS them across K.  Softmax is invariant
    to per-n shifts, so subtracting the K-mean of the logits (a rank-1
    update folded into the weights on host) bounds the logits to ~+-16 and
    removes the need for a per-n max reduction entirely.
  * x is pre-cast to fp16 on host (|x| ~ 5, rel err ~5e-4): halves the HBM
    read traffic (the memory roofline) and runs the PE at 1 cycle/row with
    cheap 2-byte weight loads (the baseline's f32r matmuls spent ~2x their
    stream time on 4-byte weight loads).
  * PACKED layout: K=64 only fills half the partitions, so each 512-column
    PSUM bank holds TWO consecutive 512-column blocks of n, one at
    partitions 0:64 (PE tile_position (0,0)) and one at 64:128 (tile
    position (0,64)).  Everything downstream of the distance GEMM (exp,
    denominator matmul, reciprocal, normalize-multiply, store) then runs at
    full 128-partition width, i.e. half the engine columns.
  * The host also folds a uniform -5 shift into the per-cluster constant
    (softmax-invariant) so exp() outputs fit fp16: the denominator matmul
    then runs at the PE's fast 16-bit rate and the normalize multiply
    reads a 2-byte operand.
  * Per core, per 2048-column super-tile of x (squares split between
    ScalarE and GpSimd so the DVE keeps capacity for the normalize):
       xsq  = x*x                              (ScalarE 1232 cols +
                                                GpSimd 816 cols, fp16)
       pd   = w1^T @ xsq + w2^T @ x            (TensorE, 8 packed matmuls
                                                into 2 PSUM banks)
       e    = exp(pd + cc - 5)                 (ScalarE, one [128,1024] op,
                                                fp16 out)
       pb   = ones_blockdiag^T @ e             (TensorE: per-block K-sum,
                                                broadcast to 64 partitions)
       r    = 1/pb                             (DVE fast approx reciprocal;
                                                sum >= 64*e^-5 > 0 always
                                                since logits are K-centered)
       out  = e * r -> fp16                    (DVE)
    x loads are [128, 4096] (two supers) to halve Sync-sequencer dispatch
    cost; stores write both packed partition halves of two supers with two
    DMAs via a rearranged DRAM access pattern.  Emission uses explicit
    keys: loads/squares keep a ~2-super lead over the GEMM but the
    schedule starts dense so the first exp is not gated on megabytes of
    DMA.
"""

import numpy as np

import concourse.bacc as bacc
import concourse.bass as bass
import concourse.tile as tile
from concourse import mybir
from concourse.bass_utils import run_bass_kernel_spmd

B, D, N, K = 16, 128, 16384, 64
NCORES = 8
BPC = B // NCORES  # batches per core
NT = 512           # one packed block (half a PSUM bank partition-wise)
SUP = 4 * NT       # compute super-tile: 2048 columns -> 2 packed PSUM banks
LSUP = 2 * SUP     # IO super-tile: one x load / one output store per 4096
SQ_ACT = 1232      # per-super square columns on ScalarE (rest on GpSimd)

F32 = mybir.dt.float32
F32R = mybir.dt.float32r
F16 = mybir.dt.float16
BF16 = mybir.dt.bfloat16

_CACHE = {}


def _build_nc():
    # Bacc (not raw Bass): its compile() pass legalizes Tile's multi-wait
    # instructions down to the 1-wait-per-instruction hardware limit.
    nc = bacc.Bacc("TRN2", target_bir_lowering=False, debug=False)
    x_in = nc.declare_dram_parameter("x", [BPC, D, N], F16, isOutput=False)
    w1_in = nc.declare_dram_parameter("w1", [D, K], F16, isOutput=False)
    w2_in = nc.declare_dram_parameter("w2", [D, K], F16, isOutput=False)
    cc_in = nc.declare_dram_parameter("cc", [2 * K, 1], F32, isOutput=False)
    ones_in = nc.declare_dram_parameter("ones_bd", [2 * K, 2 * K], F16, isOutput=False)
    # fp16 output halves the store traffic at ~5e-4 rounding (posteriors
    # live in [0,1]); the host widens back to fp32
    out_ext = nc.declare_dram_parameter("out", [BPC, K, N], F16, isOutput=True)

    with tile.TileContext(nc) as tc:
        with (
            tc.tile_pool(name="consts", bufs=1) as consts,
            tc.tile_pool(name="xp", bufs=5) as xp,
            tc.tile_pool(name="ep", bufs=8) as ep,
            tc.tile_pool(name="op", bufs=6) as op,
            tc.tile_pool(name="rp", bufs=8) as rp,
            tc.tile_pool(name="pd", bufs=2, space="PSUM") as pdp,
            tc.tile_pool(name="pb", bufs=2, space="PSUM") as pbp,
        ):
            # const tiles allocated up front; their DMAs are emitted from
            # the schedule AFTER the first two x loads so the input chain
            # (load -> square -> dist) starts ~4us earlier
            w1_sb = consts.tile([D, K], F16)
            w2_sb = consts.tile([D, K], F16)
            cc_sb = consts.tile([2 * K, 1], F32)
            ones_bd = consts.tile([2 * K, 2 * K], F16)

            def s_consts(i):
                nc.sync.dma_start(out=w1_sb, in_=w1_in[:, :])
                nc.sync.dma_start(out=w2_sb, in_=w2_in[:, :])
                nc.sync.dma_start(out=cc_sb, in_=cc_in[:, :])
                nc.sync.dma_start(out=ones_bd, in_=ones_in[:, :])

            def s_warm(i):
                # ~2us of short throwaway matmuls while the first x load is
                # in flight: ramps the PE p-state before the first dist
                # batch.  Sized to FINISH before the first xsq is ready
                # (~12us) - an earlier 18-matmul version blocked the first
                # dist in the PE FIFO and regressed.
                wt = pdp.tile([2 * K, 2 * NT], F32, tag="pd")
                for _ in range(6):
                    nc.tensor.matmul(
                        wt[0:K, 0 : 2 * K], w1_sb[:, :], ones_bd[:, :],
                        start=True, stop=True,
                    )

            n_sup = N // SUP  # 8 per batch row
            pairs = [(b, p) for b in range(BPC) for p in range(n_sup)]
            NP = len(pairs)
            st = [dict() for _ in range(NP)]

            # software-pipelined emission: each engine's in-order stream
            # interleaves stages of consecutive pairs so no stage
            # head-of-line-blocks the next pair's earlier stage
            def s0_load(i):
                # one [128, 4096] load feeds two compute super-tiles (fewer
                # 565 ns dispatches on the Sync sequencer); the first two
                # supers use small [128, 2048] loads so the pipeline's first
                # exp is not gated on megabytes of DMA
                b, p = pairs[i]
                n0 = p * SUP
                if i < 2:
                    xt = xp.tile([D, SUP], F16, tag="xt0")
                    nc.sync.dma_start(out=xt, in_=x_in[b, :, n0 : n0 + SUP])
                    st[i]["xt"] = xt
                    st[i]["xt_off"] = (xt, 0)
                elif i % 2 == 0:
                    xt = xp.tile([D, LSUP], F16, tag="xt")
                    # two half-tile DMAs: the first super's square becomes
                    # ready ~1.5us earlier and the per-queue burst halves
                    nc.sync.dma_start(out=xt[:, 0:SUP], in_=x_in[b, :, n0 : n0 + SUP])
                    nc.sync.dma_start(out=xt[:, SUP:LSUP], in_=x_in[b, :, n0 + SUP : n0 + LSUP])
                    st[i]["xt"] = xt[:, 0:SUP]
                    st[i]["xt_off"] = (xt, 0)
                else:
                    xt = st[i - 1]["xt_off"][0]
                    st[i]["xt"] = xt[:, SUP:LSUP]
                    st[i]["xt_off"] = (xt, SUP)

            def s1_square(i):
                # per-super square split between ScalarE (SQ_ACT cols) and
                # GpSimd (rest); the DVE carries reciprocal+normalize and
                # cannot also absorb the square
                xt, off = st[i]["xt_off"]
                if i < 2:
                    xsq = xp.tile([D, SUP], F16, tag="xsq0")
                    st[i]["xsq"] = xsq
                elif i % 2 == 0:
                    xsq = xp.tile([D, LSUP], F16, tag="xsq")
                    st[i]["xsq"] = xsq[:, 0:SUP]
                    st[i]["xsq_f"] = xsq
                else:
                    xsq = st[i - 1]["xsq_f"]
                    st[i]["xsq"] = xsq[:, SUP:LSUP]
                sq_a = 1344 if i < 2 else SQ_ACT
                nc.scalar.activation(
                    out=xsq[:, off : off + sq_a],
                    in_=xt[:, off : off + sq_a],
                    func=mybir.ActivationFunctionType.Square,
                )
                nc.gpsimd.tensor_mul(
                    xsq[:, off + sq_a : off + SUP],
                    xt[:, off + sq_a : off + SUP],
                    xt[:, off + sq_a : off + SUP],
                )

            def s2_dist(i):
                # pair-batched packed dist GEMM: one w1 residency covers 8
                # matmuls (both supers of the load pair), then one w2
                # residency - fewer stationary-weight switches and longer
                # uninterrupted PE runs (keeps the PE p-state high)
                if i % 2:
                    return
                pds = []
                for j in (i, i + 1):
                    pd_t = pdp.tile([2 * K, 2 * NT], F32, tag="pd")
                    st[j]["pd"] = pd_t
                    pds.append(j)
                if i == 0:
                    # first pair runs super-major so pd(0) completes without
                    # waiting on super 1's square: faster first exp
                    whj = [
                        (w, h, j)
                        for j in pds
                        for w in (w1_sb, w2_sb)
                        for h in range(2)
                    ]
                else:
                    whj = [
                        (w, h, j)
                        for w in (w1_sb, w2_sb)
                        for h in range(2)
                        for j in pds
                    ]
                # h (the PE tile position / PSUM partition half) varies
                # OUTSIDE g and j: consecutive matmuls keep the same
                # stationary weight AND position, so the PE skips the
                # ~100-177ns weight reload between them
                for w_sb, h, j in whj:
                    start = w_sb is w1_sb
                    src_t = st[j]["xsq"] if start else st[j]["xt"]
                    pd_t = st[j]["pd"]
                    for g in range(2):
                        c0 = g * 2 * NT + h * NT
                        nc.tensor.matmul(
                            pd_t[h * K : (h + 1) * K, g * NT : (g + 1) * NT],
                            w_sb[:, :],
                            src_t[:, c0 : c0 + NT],
                            start=start,
                            stop=not start,
                        )

            def s3_exp(i):
                pd_t = st[i].pop("pd")
                et = ep.tile([2 * K, 2 * NT], F16, tag="et")
                # one 1024-col op across both PSUM banks; bias is the packed
                # per-cluster constant (duplicated across the two halves)
                nc.scalar.activation(
                    out=et, in_=pd_t,
                    func=mybir.ActivationFunctionType.Exp,
                    bias=cc_sb, scale=1.0,
                )
                st[i]["et"] = et
                st[i].pop("xt")
                st[i].pop("xsq")

            def s4_den(i):
                et = st[i]["et"]
                # denominator: block-diagonal ones weight sums each packed
                # 64-partition block AND broadcasts the sum back to all 64
                # of its partitions in a single pass
                pb_t = pbp.tile([2 * K, 2 * NT], F32, tag="pb")
                for g in range(2):
                    sl = slice(g * NT, (g + 1) * NT)
                    nc.tensor.matmul(
                        pb_t[:, sl], ones_bd[:, :], et[:, sl],
                        start=True, stop=True,
                    )
                st[i]["pb"] = pb_t

            def s5_recip(i):
                pb_t = st[i].pop("pb")
                r_all = rp.tile([2 * K, 2 * NT], F32, tag="r")
                # ~18-bit-accurate custom-DVE reciprocal; the sum is always
                # >= 64*e^{-16} (K-centered logits), so the undefined edge
                # cases (0/denorm/inf) cannot occur
                nc.vector.reciprocal_approx_fast(out=r_all, in_=pb_t)
                st[i]["r"] = r_all

            def s6_mult(i):
                et, r_all = st[i].pop("et"), st[i].pop("r")
                if i % 2:
                    ot = st[i - 1]["ot_full"]
                    ov = ot[:, 2 * NT : 4 * NT]
                else:
                    ot = op.tile([2 * K, 4 * NT], F16, tag="ot")
                    st[i]["ot_full"] = ot
                    ov = ot[:, 0 : 2 * NT]
                nc.vector.tensor_mul(ov, et, r_all)

            def s7_store(i):
                # store once per two compute super-tiles; issued from the
                # GpSimd sequencer (cheap SWDGE dispatch) to keep the Sync
                # sequencer free for the x loads
                if i % 2 == 0:
                    return
                b, p = pairs[i]
                n0 = (p - 1) * SUP
                ot = st[i - 1].pop("ot_full")
                # DRAM view [h][k][q][c] <-> packed SBUF [h*64+k, q*512+c],
                # n = n0 + q*1024 + h*512 + c
                d4 = out_ext[b, :, n0 : n0 + LSUP].rearrange(
                    "k (q h c) -> h k q c", q=4, h=2
                )
                nc.sync.dma_start(out=d4[0], in_=ot[0:K, :])
                nc.sync.dma_start(out=d4[1], in_=ot[K : 2 * K, :])

            stages = [
                s0_load, s1_square, s2_dist, s3_exp,
                s4_den, s5_recip, s6_mult, s7_store,
            ]
            # explicit emission keys: loads/squares hold a steady ~2-super
            # lead over the GEMM (keeps every FIFO fed) but the schedule
            # starts dense, so the first exp is behind only two small
            # squares/loads instead of five 1 MB ones
            LEAD = [-4.15, -3.85, 0.0, 0.2, 0.3, 0.4, 0.5, 0.6]
            sched = []
            for i in range(NP):
                for k in range(len(stages)):
                    sched.append((i + LEAD[k], k, i))
            stages.append(s_consts)
            sched.append((-3.0, len(stages) - 1, 0))
            stages.append(s_warm)
            sched.append((-2.9, len(stages) - 1, 0))
            for _, k, i in sorted(sched):
                stages[k](i)
    nc.compile()
    return nc


def _host_params(mu, log_sigma, log_alpha):
    mu64 = mu.astype(np.float64)
    mu_n = mu64 / np.maximum(
        np.linalg.norm(mu64, axis=1, keepdims=True), 1e-12
    )
    sinv = np.exp(-log_sigma.astype(np.float64))  # (K, D)
    a1 = -sinv                                    # coeff of x^2 in logits
    a2 = 2.0 * mu_n * sinv                        # coeff of x
    c = (
        -np.sum(mu_n * mu_n * sinv, axis=1)
        + log_alpha.astype(np.float64)
        - 0.5 * np.sum(log_sigma.astype(np.float64), axis=1)
    )
    # center across K: softmax is invariant to per-n shifts, and this keeps
    # the on-device logits within exp()'s comfortable range (~+-16)
    a1c = a1 - a1.mean(axis=0, keepdims=True)
    a2c = a2 - a2.mean(axis=0, keepdims=True)
    ccv = c - c.mean() - 5.0
    w1 = np.ascontiguousarray(a1c.T, dtype=np.float16)  # (D, K)
    w2 = np.ascontiguousarray(a2c.T, dtype=np.float16)  # (D, K)
    cc = np.tile(ccv.astype(np.float32).reshape(K, 1), (2, 1))  # (128, 1)
    return w1, w2, cc


def _in_maps(x, mu, log_sigma, log_alpha):
    x = np.asarray(x).astype(np.float16)
    w1, w2, cc = _host_params(
        np.asarray(mu), np.asarray(log_sigma), np.asarray(log_alpha)
    )
    ones_bd = np.kron(np.eye(2), np.ones((K, K))).astype(np.float16)
    return [
        {
            "x": np.ascontiguousarray(x[i * BPC : (i + 1) * BPC]),
            "w1": w1,
            "w2": w2,
            "cc": cc,
            "ones_bd": ones_bd,
        }
        for i in range(NCORES)
    ]


def kernel(x, mu, log_sigma, log_alpha):
    if "nc" not in _CACHE:
        _CACHE["nc"] = _build_nc()
    nc = _CACHE["nc"]
    in_maps = _in_maps(x, mu, log_sigma, log_alpha)
    res = run_bass_kernel_spmd(nc, in_maps, list(range(NCORES))).results
    out = np.concatenate(
        [np.asarray(res[i]["out"]) for i in range(NCORES)], axis=0
    )
    return out.astype(np.float32)
